# revision 5
# baseline (speedup 1.0000x reference)
"""Trainium2 Bass kernel for nn_Attention_50637664420407 — v2.

Dense causal transformer block: LayerNorm -> QKV -> RoPE -> causal attention
-> out-projection.  x:[2,2048,1024] f32.

Sharding (8 cores): head-parallel.  Core c owns heads {2c, 2c+1} for both
batch elements.  W_qkv is split column-wise per head group, W_out row-wise;
each core computes a full [4096,1024] partial of the output projection (bf16)
and the host sums the 8 partials.

v2 changes over the baseline:
- QKV runs as fp8e4 DoubleRow matmuls with residual compensation:
  x = x8 + dx8, 32*Wg = w8 + dw8 (all four operands e4m3, residuals keep
  full-precision error feedback), accumulating x8*w8 + dx8*w8 + x8*dw8.
  Effective precision beats bf16 (validated 2.4e-3 rel err vs 3.6e-3 for
  pure bf16) at ~0.78x the PE time of the bf16 QKV.
- LN stats matmuls also run fp8 DoubleRow from x8 (mu/var shift is O(2e-4)).
- The W-scale 32 is folded into the rstd row (exp bias) and the G/bq
  correction rows, so no extra scaling work exists anywhere.
- The LN-stats collective carries [rstd/32; std; mu] (3 rows) so the
  gathered path needs no reciprocals.
- Phase D runs I-outer AV per pair with a double-buffered av psum bank so
  consecutive pairs' chains overlap; pair p+1's scores+exp run threaded
  between pair p's AV matmuls (filler queue).  Out-projections spread over
  the whole phase with evictions on whichever of DVE/ACT is idle; late
  batch-1 tiles run after each av pass's matmuls (never head-of-line
  blocking the PE on a pending normalize) with a 3-deep tail psum rotation.
"""

import sys
import numpy as np

for _p in ("/opt/trn_rl_repo", "/root/.axon_site/_ro/trn_rl_repo"):
    if _p not in sys.path:
        sys.path.append(_p)

import concourse.bass as bass
import concourse.bacc as bacc
import concourse.mybir as mybir
import concourse.tile as tile
from concourse.alu_op_type import AluOpType

F32 = mybir.dt.float32
BF16 = mybir.dt.bfloat16
FP8 = mybir.dt.float8e4
AF = mybir.ActivationFunctionType
DR = mybir.MatmulPerfMode.DoubleRow

P = 128          # partitions
T = 4096         # total tokens (2 batches x 2048)
NT = 2048        # seq len per batch
DIMK = 1024      # model dim
KC = 8           # k chunks of 128
TT = 8           # token tiles of 512
D = 64           # head dim
SCALE = D ** -0.5  # 0.125
WSCALE = 32.0           # W_qkv prescale for fp8
LNW = float(np.log(1.0 / WSCALE))
VSCALE = 8.0            # v scaled at the vb copy (helps fp8 outproj range)
OUTPROJ_FP8 = False   # probed at 4.4e-2 rel err — fails the 2e-2 gate
OSCALE = (1.0 / (VSCALE * 32.0)) if OUTPROJ_FP8 else (1.0 / VSCALE)


def ts(i, n):
    return slice(i * n, (i + 1) * n)


class _Bacc(bacc.Bacc):
    """Bacc with a pinned ACT table-set choice (see baseline)."""

    def insert_act_table_loads(self):
        import concourse.bass_isa as bass_isa  # noqa: F401
        from concourse.hw_specs import get_activation_tables
        import bass_rust as _bass_rust
        has_activation = any(
            isinstance(i, mybir.InstActivation)
            for b in self.main_func.blocks
            for i in b.instructions
        )
        if not has_activation:
            return
        pinned = {AF.Exp, AF.Ln, AF.Square}
        keep = "natural_log_exp_and_others"
        tables = []
        for name, funcs in get_activation_tables(self.m.arch).items():
            if name != keep:
                funcs = funcs - pinned
            tables.append((name, funcs))
        _bass_rust.insert_act_table_loads(self, tables)


def build_program():
    nc = _Bacc("TRN2", target_bir_lowering=False, debug=False, num_devices=8)

    xi8_h = nc.declare_dram_parameter("xi8", [P, KC, 2, T], FP8, False)
    w8_h = nc.declare_dram_parameter("w8", [P, KC, 384], FP8, False)
    dw8_h = nc.declare_dram_parameter("dw8", [P, KC, 384], FP8, False)
    wo_h = (nc.declare_dram_parameter("wo", [D, 2, DIMK], FP8, False)
            if OUTPROJ_FP8 else
            nc.declare_dram_parameter("wo", [P, DIMK], BF16, False))
    gb_h = nc.declare_dram_parameter("gb", [2, 384], BF16, False)  # [32bq;-32G]
    cc_h = nc.declare_dram_parameter("cc", [P, NT], BF16, False)
    ss_h = nc.declare_dram_parameter("ss", [P, NT], BF16, False)
    tri_h = nc.declare_dram_parameter("tri", [P, P], BF16, False)
    perm_h = nc.declare_dram_parameter("perm", [P, P], BF16, False)
    out_h = nc.declare_dram_parameter("out", [T, DIMK], BF16, True)
    xs8_h = nc.declare_dram_parameter("xs8", [P, KC * 512], FP8, False)
    sl_h = nc.dram_tensor("stats_loc", [3, 512], BF16)   # [rstd/32; std; mu]
    sg_h = nc.dram_tensor("stats_all", [24, 512], BF16)

    with tile.TileContext(nc) as tc:
        with tc.tile_pool(name="const", bufs=1) as const, \
             tc.tile_pool(name="qkvsb", bufs=1) as qkvsb, \
             tc.tile_pool(name="ohp", bufs=1) as ohp, \
             tc.tile_pool(name="pp", bufs=2) as pp, \
             tc.tile_pool(name="vbp", bufs=2) as vbp, \
             tc.tile_pool(name="stp", bufs=2, space="PSUM") as stp:

            qkvn = qkvsb.tile([P, 3, T], BF16)
            qn = qkvn[:, 0, :]
            kn = qkvn[:, 1, :]
            vn = qkvn[:, 2, :]
            if OUTPROJ_FP8:
                oh8 = ohp.tile([D, 2, T], FP8)   # attn out, 8*oh, head-grouped
            else:
                ohT = ohp.tile([P, T], BF16)

            const_tiles = {}

            def load_consts_early():
                # cc/ss arrive in quarters interleaved with the x stream
                # (rope(t) only needs quarter t%4); perm before rope(0)
                cc_t = const.tile([P, NT], BF16)
                ss_t = const.tile([P, NT], BF16)
                perm_t = const.tile([P, P], BF16)
                ident = const.tile([P, P], BF16)
                nc.gpsimd.memset(ident, 0.0)
                nc.gpsimd.affine_select(out=ident, in_=ident,
                                        compare_op=AluOpType.not_equal, fill=1.0,
                                        base=0, pattern=[[-1, P]],
                                        channel_multiplier=1)
                eps1 = const.tile([1, 1], F32)
                nc.vector.memset(eps1, 1e-5)
                elnw = const.tile([1, 1], F32)
                nc.vector.memset(elnw, LNW)
                ones8 = const.tile([P, 2, 32], FP8)
                nc.vector.memset(ones8, 1.0)
                const_tiles.update(ident=ident, eps1=eps1, elnw=elnw,
                                   perm=perm_t, cc=cc_t, ss=ss_t, ones8=ones8)

            def load_w():
                # early on the SP queue: tile 0's dw pass needs these by ~6us
                w_t = const.tile([P, KC, 384], FP8)
                dw_t = const.tile([P, KC, 384], FP8)
                nc.sync.dma_start(out=w_t, in_=w8_h[:, :, :])
                nc.sync.dma_start(out=dw_t, in_=dw8_h[:, :, :])
                gb_t = const.tile([2, 384], BF16)
                nc.sync.dma_start(out=gb_t, in_=gb_h[:, :])
                const_tiles.update(w=w_t, dw=dw_t, gb=gb_t)

            def load_consts_late():
                tri_t = const.tile([P, P], BF16)
                nc.gpsimd.dma_start(out=tri_t, in_=tri_h[:, :])
                if OUTPROJ_FP8:
                    wo_t = const.tile([D, 2, DIMK], FP8)
                    nc.gpsimd.dma_start(out=wo_t, in_=wo_h[:, :, :])
                else:
                    wo_t = const.tile([P, DIMK], BF16)
                    nc.gpsimd.dma_start(out=wo_t, in_=wo_h[:, :])
                const_tiles.update(tri=tri_t, wo=wo_t)

            # ---------------- phase D emit helpers (also used in phase A) ---
            ptiles = {}   # pair -> list of 16 exp'd score tiles
            vbs = {}      # pair -> v token-major tile (8*v)

            def emit_scores_J(pair, J):
                b, h = pair // 2, pair % 2
                base, hr = NT * b, D * h
                m = J % 4
                i0 = 512 * (J // 4)
                ilen = NT - i0
                pJ = pp.tile([P, ilen], BF16, tag=f"p{J}", name=f"p{J}_{pair}")
                ptiles.setdefault(pair, {})[J] = pJ
                lhs = kn[hr : hr + D, base + P * J : base + P * (J + 1)]
                for c0 in range(0, ilen, 1024):
                    clen = min(1024, ilen - c0)
                    st = stp.tile([P, 1024], F32, tag="st")
                    off0 = P * m if c0 == 0 else 0
                    starts = [off0] if off0 else []
                    starts += list(range(512 if off0 else 0, clen, 512))
                    for boff in starts:
                        n = min(512 - (boff % 512), clen - boff)
                        nc.tensor.matmul(
                            st[:, boff : boff + n],
                            lhsT=lhs,
                            rhs=qn[hr : hr + D,
                                   base + i0 + c0 + boff :
                                   base + i0 + c0 + boff + n],
                            start=True, stop=True)
                    nc.scalar.activation(out=pJ[:, c0 + off0 : c0 + clen],
                                         in_=st[:, off0:clen],
                                         func=AF.Exp, scale=SCALE)
                # causal mask on the diagonal 128x128 sub-block
                nc.vector.tensor_mul(pJ[:, P * m : P * (m + 1)],
                                     pJ[:, P * m : P * (m + 1)],
                                     const_tiles["tri"])

            def emit_vtrans(pair, g):
                b, h = pair // 2, pair % 2
                base, hr = NT * b, D * h
                ident = const_tiles["ident"]
                if g == 0:
                    vb = vbp.tile([P, 16, D + 1], BF16, tag="vb",
                                  name=f"vb_{pair}")
                    vbs[pair] = vb
                    nc.gpsimd.memset(vb[:, :, D : D + 1], 1.0)
                vb = vbs[pair]
                tp = stp.tile([P, 2048], BF16, tag="st")
                for jj in range(8):
                    J8 = 8 * g + jj
                    nc.tensor.transpose(
                        tp[:, D * jj : D * (jj + 1)],
                        in_=vn[hr : hr + D, base + P * J8 : base + P * (J8 + 1)],
                        identity=ident[hr : hr + D, hr : hr + D])
                # v -> 8*v keeps the fp8 outproj inputs out of subnormals
                nc.vector.tensor_scalar_mul(vb[:, ts(g, 8), 0:D],
                                            in0=tp[:, 0 : 8 * D],
                                            scalar1=VSCALE)

            pools = {}

            def emit_av_I(pair, I, fillers=()):
                """I-outer AV: accumulate all key tiles J<=4I+3 into av_I,
                then normalize.  One av psum bank, double-buffered, so
                consecutive pairs' chains overlap.  `fillers` are emission
                thunks (scores/outproj) threaded between the av matmuls so
                the PE never stalls on the st-psum/exp recycle."""
                b, h = pair // 2, pair % 2
                base, hr = NT * b, D * h
                vb = vbs[pair]
                fillers = list(fillers)
                av = pools["avp"].tile([D + 1, 512], F32, tag="av",
                                       name=f"av{I}_{pair}")
                for J in range(4 * I + 4):
                    if J % 3 == 1 and fillers:
                        fillers.pop(0)()
                    pJ = ptiles[pair][J]
                    i0 = 512 * (J // 4)
                    cbase = 512 * I - i0
                    off = P * (J % 4) if J // 4 == I else 0
                    nc.tensor.matmul(
                        av[0 : D + 1, off:512],
                        lhsT=vb[:, J, :],
                        rhs=pJ[:, cbase + off : cbase + 512],
                        start=(J == 0), stop=(J == 4 * I + 3))
                for f in fillers:
                    f()
                rec = pools["recp"].tile([1, 512], F32, tag="rec")
                nc.vector.reciprocal(rec, av[D : D + 1, 0:512])
                rb2 = pools["recp"].tile([D, 512], F32, tag="rb2")
                nc.gpsimd.partition_broadcast(rb2, rec, channels=D)
                if OUTPROJ_FP8:
                    dst = oh8[:, h, base + 512 * I : base + 512 * (I + 1)]
                else:
                    dst = ohT[hr : hr + D,
                              base + 512 * I : base + 512 * (I + 1)]
                nc.vector.tensor_mul(dst, av[0:D, 0:512], rb2)

            def emit_outproj_tile(t, eng):
                wo_t = const_tiles["wo"]
                if (t % 3 == 2) if t >= 24 else (t % 2 == 0):
                    # tail tiles rotate 3-deep (st,st,op): the scores are done
                    # so both st buffers are free for the outproj pipeline
                    op_ps = pools["opps"].tile([P, 1024], F32, tag="op",
                                               name="op_ps")
                else:
                    op_ps = stp.tile([P, 1024], F32, tag="st", name="op_ps")
                for cb in range(2):
                    if OUTPROJ_FP8:
                        nc.tensor.matmul(op_ps[:, ts(cb, 512)],
                                         lhsT=oh8[:, :, ts(t, P)],
                                         rhs=wo_t[:, :, ts(cb, 512)],
                                         start=True, stop=True,
                                         perf_mode=DR)
                    else:
                        nc.tensor.matmul(op_ps[:, ts(cb, 512)],
                                         lhsT=ohT[:, ts(t, P)],
                                         rhs=wo_t[:, ts(cb, 512)],
                                         start=True, stop=True)
                ev = pools["outep"].tile([P, DIMK], BF16, tag="ev")
                if eng == "v":
                    nc.vector.tensor_scalar_mul(ev, in0=op_ps, scalar1=OSCALE)
                elif eng == "s":
                    nc.scalar.mul(ev, op_ps, OSCALE)
                else:   # split halves across DVE+ACT (gpsimd can't read PSUM)
                    nc.vector.tensor_scalar_mul(ev[:, 0:512],
                                                in0=op_ps[:, 0:512],
                                                scalar1=OSCALE)
                    nc.scalar.mul(ev[:, 512:1024], op_ps[:, 512:1024], OSCALE)
                nc.sync.dma_start(out=out_h[ts(t, P), :], in_=ev)

            # ---------- phases A-C: stats + QKV + RoPE, per 512-token tile;
            # batch-0 scores/exp interleaved into tiles 4-7 ----------
            with tc.tile_pool(name="stg", bufs=2) as stg, \
                 tc.tile_pool(name="stg1", bufs=1) as stg1, \
                 tc.tile_pool(name="xtc", bufs=3) as xtc, \
                 tc.tile_pool(name="xsq", bufs=4) as xsqp, \
                 tc.tile_pool(name="murp", bufs=2) as murp, \
                 tc.tile_pool(name="statsg", bufs=1) as statsg, \
                 tc.tile_pool(name="xsp", bufs=1) as xsp, \
                 tc.tile_pool(name="qkps", bufs=1, space="PSUM") as qkps, \
                 tc.tile_pool(name="stps", bufs=1, space="PSUM") as stps:

                xtiles = {}

                def fetch_x(t, split=False):
                    xt = xtc.tile([P, KC, 2, 512], FP8, tag="x", name=f"x_{t}")
                    if split:
                        # first chunk-pair lands early so QKV starts sooner
                        nc.sync.dma_start(out=xt[:, 0:2, :, :],
                                          in_=xi8_h[:, 0:2, :, ts(t, 512)])
                        nc.sync.dma_start(out=xt[:, 2:KC, :, :],
                                          in_=xi8_h[:, 2:KC, :, ts(t, 512)])
                    else:
                        nc.sync.dma_start(out=xt, in_=xi8_h[:, :, :, ts(t, 512)])
                    xtiles[t] = xt

                xss_t = xsp.tile([P, KC, 512], FP8, name="xss_t")
                xsv = xs8_h.rearrange("p (k c) -> p k c", k=KC)
                nc.sync.dma_start(out=xss_t[:, 0:2, :], in_=xsv[:, 0:2, :])
                nc.sync.dma_start(out=xss_t[:, 2:KC, :], in_=xsv[:, 2:KC, :])
                fetch_x(0)
                load_w()
                load_consts_early()
                cc_t0 = const_tiles["cc"]
                ss_t0 = const_tiles["ss"]
                fetch_x(1)
                nc.sync.dma_start(out=cc_t0[:, 0:512], in_=cc_h[:, 0:512])
                nc.sync.dma_start(out=ss_t0[:, 0:512], in_=ss_h[:, 0:512])
                nc.sync.dma_start(out=const_tiles["perm"], in_=perm_h[:, :])
                fetch_x(2)
                nc.sync.dma_start(out=cc_t0[:, 512:1024],
                                  in_=cc_h[:, 512:1024])
                nc.sync.dma_start(out=ss_t0[:, 512:1024],
                                  in_=ss_h[:, 512:1024])
                murt = {}
                gb_t = const_tiles["gb"]
                cc_t = const_tiles["cc"]
                ss_t = const_tiles["ss"]
                perm_t = const_tiles["perm"]
                eps1 = const_tiles["eps1"]
                elnw = const_tiles["elnw"]
                ones8 = const_tiles["ones8"]

                dsts = [qn, kn, vn]

                def emit_qkv_pass(qkv_ps, c, xop, plane, wop, first):
                    for k2 in range(KC // 2):
                        k_sl = slice(2 * k2, 2 * k2 + 2)
                        rhs = (xop[:, k_sl, plane, :] if plane is not None
                               else xop[:, k_sl, :])
                        nc.tensor.matmul(
                            qkv_ps[:, c, :],
                            lhsT=wop[:, k_sl, ts(c, P)],
                            rhs=rhs,
                            start=(first and k2 == 0),
                            stop=False,
                            perf_mode=DR)

                def stats_s1_sq(s12, xt, plane):
                    """S1 ones-matmuls + square fills.  The S2 matmuls are
                    emitted separately (stats_s2) after ~2 QKV passes so the
                    in-order PE queue never waits on the DVE/ACT squares."""
                    def pair_ap(k2):
                        k_sl = slice(2 * k2, 2 * k2 + 2)
                        return (xt[:, k_sl, plane, :] if plane is not None
                                else xt[:, k_sl, :])

                    def chunk_ap(k):
                        return (xt[:, k, plane, :] if plane is not None
                                else xt[:, k, :])

                    for k2 in range(KC // 2):
                        nc.tensor.matmul(s12[0:32, :], lhsT=ones8,
                                         rhs=pair_ap(k2),
                                         start=(k2 == 0), stop=(k2 == KC // 2 - 1),
                                         perf_mode=DR)
                    sqs = []
                    for k2 in range(KC // 2):
                        sq = xsqp.tile([P, 2, 512], FP8, tag="sq")
                        for i in range(2):
                            if (k2 + i) % 2 == 0:
                                nc.vector.tensor_mul(sq[:, i, :],
                                                     chunk_ap(2 * k2 + i),
                                                     chunk_ap(2 * k2 + i))
                            else:
                                nc.scalar.square(sq[:, i, :],
                                                 chunk_ap(2 * k2 + i))
                        sqs.append(sq)
                    return sqs

                def stats_s2(s12, sqs):
                    for k2, sq in enumerate(sqs):
                        nc.tensor.matmul(s12[0:32, :], lhsT=ones8,
                                         rhs=sq,
                                         start=(k2 == 0), stop=(k2 == KC // 2 - 1),
                                         perf_mode=DR)

                # ---- sharded LN stats: this core's 512 tokens only.
                # Tile 0's first QKV pass is hoisted between S1 and S2 so the
                # PE isn't queue-blocked on the square fills.
                s12o = stps.tile([32, 512], F32, tag="s12", name="s12o")
                sqs_o = stats_s1_sq(s12o, xss_t, None)
                # mu must be read out BEFORE the S2 matmuls overwrite rows 0-31
                muo = stg1.tile([1, 512], BF16, tag="mu", name="muo")
                nc.vector.tensor_scalar_mul(muo, in0=s12o[0:1, :],
                                            scalar1=1.0 / DIMK)
                nc.scalar.dma_start(out=sl_h[2:3, :], in_=muo)
                t2o = stg1.tile([1, 512], F32, tag="t2", name="t2o")
                nc.vector.tensor_mul(t2o, muo, muo)
                qkv_ps0 = qkps.tile([P, 3, 512], F32, tag="qkv",
                                    name="qkv_ps")
                emit_qkv_pass(qkv_ps0, 0, xtiles[0], 0, const_tiles["w"], True)
                stats_s2(s12o, sqs_o)
                lvo = stg1.tile([1, 512], F32, tag="lv", name="lvo")
                nc.vector.scalar_tensor_tensor(out=lvo, in0=s12o[0:1, :],
                                               scalar=1.0 / DIMK, in1=t2o,
                                               op0=AluOpType.mult,
                                               op1=AluOpType.subtract)
                nc.scalar.activation(out=lvo, in_=lvo, func=AF.Ln, bias=eps1)
                sdo = stg1.tile([1, 512], BF16, tag="sd", name="sdo")
                nc.scalar.activation(out=sdo, in_=lvo, func=AF.Exp, scale=0.5)
                nc.scalar.dma_start(out=sl_h[1:2, :], in_=sdo)
                rso = stg1.tile([1, 512], BF16, tag="rso", name="rso")
                nc.scalar.activation(out=rso, in_=lvo, func=AF.Exp,
                                     scale=-0.5, bias=elnw)
                nc.scalar.dma_start(out=sl_h[0:1, :], in_=rso)
                nc.gpsimd.collective_compute(
                    "AllGather", mybir.AluOpType.bypass,
                    replica_groups=[[0, 1, 2, 3, 4, 5, 6, 7]],
                    ins=[sl_h[:, :]], outs=[sg_h[:, :]])
                w_t = const_tiles["w"]
                dw_t = const_tiles["dw"]
                load_consts_late()
                # gathered stats land via rearranged DMAs on the ACT queue
                # (idle during tiles 2-7): [std; mu] rows for the correction
                # matmul (base partition 0), rstd/32 separately for broadcasts
                sall = statsg.tile([2, T], BF16, name="sall")
                rall = statsg.tile([1, T], BF16, name="rall")
                rbs = {}

                def rb_chain(t):
                    rb = stg.tile([P, 512], BF16, tag="rb", name=f"rb{t}")
                    nc.gpsimd.partition_broadcast(rb, rall[0:1, ts(t, 512)],
                                                  channels=P)
                    rbs[t] = rb

                lst = {}

                def local_stats_pre(t, xt):
                    murt[t] = murp.tile([2, 512], BF16, tag="mur",
                                        name=f"mur_{t}")
                    s12 = stps.tile([32, 512], F32, tag="s12",
                                    name=f"s12_{t}")
                    sqs = stats_s1_sq(s12, xt, 0)
                    mu = stg1.tile([1, 512], BF16, tag="mu")
                    nc.vector.tensor_scalar_mul(mu, in0=s12[0:1, :],
                                                scalar1=1.0 / DIMK)
                    nc.scalar.dma_start(out=murt[t][1:2, :], in_=mu)
                    t2 = stg1.tile([1, 512], F32, tag="t2")
                    nc.vector.tensor_mul(t2, mu, mu)
                    lst[t] = (s12, sqs, t2)

                def local_stats_post(t):
                    s12, sqs, t2 = lst.pop(t)
                    stats_s2(s12, sqs)
                    lv = stg1.tile([1, 512], F32, tag="lv")
                    nc.vector.scalar_tensor_tensor(out=lv, in0=s12[0:1, :],
                                                   scalar=1.0 / DIMK,
                                                   in1=t2,
                                                   op0=AluOpType.mult,
                                                   op1=AluOpType.subtract)
                    nc.scalar.activation(out=lv, in_=lv, func=AF.Ln,
                                         bias=eps1)
                    rs = stg.tile([1, 512], F32, tag="rs")
                    nc.scalar.activation(out=rs, in_=lv, func=AF.Exp,
                                         scale=-0.5, bias=elnw)
                    nc.scalar.activation(out=murt[t][0:1, :], in_=lv,
                                         func=AF.Exp, scale=0.5)
                    rb = stg.tile([P, 512], F32, tag="rb", name=f"rbl{t}")
                    nc.gpsimd.partition_broadcast(rb, rs, channels=P)
                    rbs[t] = rb

                def emit_rope(t):
                    # RoPE in place on q, k: rotate-half via PE permutation.
                    # Called from tile t+1's body so the perm matmuls never
                    # head-of-line block the PE queue on the evict writes.
                    cs = ts(t % 4, 512)
                    for ci, src in enumerate((qn, kn)):
                        sl = src[:, ts(t, 512)]
                        rp = stp.tile([P, 1024], F32, tag="st", name="rp")
                        nc.tensor.matmul(rp[:, 0:512], lhsT=perm_t, rhs=sl,
                                         start=True, stop=True)
                        ra = stg.tile([P, 512], BF16, tag="ra")
                        nc.gpsimd.tensor_mul(ra, sl, cc_t[:, cs])
                        rb2_ = stg.tile([P, 512], BF16, tag="rb2")
                        nc.vector.tensor_mul(rb2_, rp[:, 0:512], ss_t[:, cs])
                        nc.vector.tensor_add(sl, ra, rb2_)

                for t in range(TT):
                    if t + 3 < TT:
                        fetch_x(t + 3)
                    if t == 0:
                        nc.sync.dma_start(out=cc_t0[:, 1024:2048],
                                          in_=cc_h[:, 1024:2048])
                        nc.sync.dma_start(out=ss_t0[:, 1024:2048],
                                          in_=ss_h[:, 1024:2048])
                    xt = xtiles.pop(t)

                    # rope(t-1) + first score tile fill the PE while tile
                    # t-1's eviction drains, releasing the qkv psum; the
                    # remaining score tiles interleave between QKV passes so
                    # the PE never stalls on the st-psum/exp recycle
                    if t > 0:
                        emit_rope(t - 1)
                    sc = (list(range(4 * (t - 4), 4 * (t - 4) + 4))
                          if t >= 4 else [])
                    if sc:
                        emit_scores_J(0, sc[0])
                        emit_scores_J(0, sc[1])

                    # QKV: 36 fp8-DR chunk passes (x8*w8, dx8*w8, x8*dw8),
                    # then the 3 LN-fold correction rows, then one fused
                    # evict.  Local stats for tiles 0-2 are hoisted a tile
                    # early so their DVE/ACT squares hide under PE work.
                    qkv_ps = qkv_ps0 if t == 0 else qkps.tile(
                        [P, 3, 512], F32, tag="qkv", name="qkv_ps")
                    for c in range(3):
                        for pi, (plane, wop) in enumerate(
                                ((0, w_t), (1, w_t), (0, dw_t))):
                            if t == 0 and c == 0 and pi == 0:
                                continue    # hoisted into the preamble
                            emit_qkv_pass(qkv_ps, c, xt, plane, wop,
                                          pi == 0 and not (t == 0 and c == 0))
                        if c == 0 and t == 0:
                            local_stats_pre(0, xt)
                        if c == 1 and t < 3:
                            local_stats_post(t)
                        if c + 2 < len(sc):
                            emit_scores_J(0, sc[c + 2])
                    if t == 6:
                        emit_vtrans(0, 0)
                        emit_vtrans(0, 1)
                    if t + 1 < 3:
                        local_stats_pre(t + 1, xtiles[t + 1])
                    if t == 2:
                        # gathered stats: on the ACT queue AFTER tile 2's
                        # local chain (ACT idles from here to the first exp)
                        sgv = sg_h.rearrange("(r i) c -> i r c", i=3)
                        nc.scalar.dma_start(
                            out=sall.rearrange("i (r c) -> i r c", c=512),
                            in_=sgv[1:3])
                        nc.scalar.dma_start(
                            out=rall.rearrange("i (r c) -> i r c", c=512),
                            in_=sgv[0:1])
                    if t < 3:
                        rb_t = rbs.pop(t)
                    else:
                        murt[t] = sall[0:2, ts(t, 512)]
                        if t not in rbs:
                            rb_chain(t)
                        rb_t = rbs.pop(t)
                        if t + 1 < TT:
                            rb_chain(t + 1)
                    for c in range(3):
                        nc.tensor.matmul(qkv_ps[:, c, :],
                                         lhsT=gb_t[:, ts(c, P)],
                                         rhs=murt[t],
                                         start=False, stop=True)
                    for c in range(3):
                        nc.vector.tensor_mul(dsts[c][:, ts(t, 512)],
                                             qkv_ps[:, c, :], rb_t)
                emit_rope(TT - 1)

            # ---------- phase D: AV / remaining scores / out-projection ----
            # I-outer AV per pair, pairs overlapped via the double-buffered
            # av bank; pair p+1's scores+exp run under pair p's AV matmuls.
            with tc.tile_pool(name="avp", bufs=2, space="PSUM") as avp, \
                 tc.tile_pool(name="opps", bufs=1, space="PSUM") as opps, \
                 tc.tile_pool(name="recp", bufs=3) as recp, \
                 tc.tile_pool(name="oute", bufs=6) as outep:
                pools.update(avp=avp, opps=opps, recp=recp, outep=outep)
                # pair p's 16 score tiles spread cost-balanced over its 8
                # emission steps; each J lands just before the av pass that
                # first consumes it
                SPREAD = [[0], [1], [2, 8], [3, 9], [4, 10], [5, 6],
                          [7, 11, 12], [13, 14, 15]]
                def sc_thunk(p, j):
                    return lambda: emit_scores_J(p, j)

                def vt_thunk(p):
                    def f():
                        emit_vtrans(p, 0)
                        emit_vtrans(p, 1)
                    return f

                def op_thunk(tl, eng):
                    return lambda: emit_outproj_tile(tl, eng)

                for step in range(17):
                    fillers = []
                    post16 = []
                    if step == 16:
                        # tiles 24-27 only need av(3,2); run them AFTER the
                        # final av pass's matmuls so they don't head-of-line
                        # block it while waiting for av(3,2)'s normalize
                        for j, tl in enumerate(range(24, 28)):
                            post16.append(op_thunk(tl, ("s", "v")[j % 2]))
                    # scores for pair p at steps 4(p-1)..4(p-1)+7; pairs 2/3
                    # wait for pair p-2's probs release at its I3 pass
                    for pair in (1, 2, 3):
                        js = step - 4 * (pair - 1)
                        if 0 <= js < 8:
                            for J in SPREAD[js]:
                                fillers.append(sc_thunk(pair, J))
                            if js == 0:
                                fillers.append(vt_thunk(pair))
                    # outproj: batch-0 tiles after pair1's av_I (steps 6..9),
                    # batch-1 after pair3's av_I (steps 14..16)
                    if 7 <= step <= 10:
                        for j, tl in enumerate(range(4 * (step - 7),
                                               4 * (step - 7) + 4)):
                            fillers.append(op_thunk(tl, ("v", "b")[j % 2]))
                    if 14 <= step <= 15:
                        for j, tl in enumerate(range(16 + 4 * (step - 14),
                                               20 + 4 * (step - 14))):
                            post16.append(op_thunk(tl, ("s", "v")[j % 2]))
                    # av passes: pair p runs its I=0..3 at steps 4p+1..4p+4,
                    # threading this step's fillers between its matmuls
                    done = False
                    for pair in range(4):
                        I = step - 4 * pair - 1
                        if 0 <= I < 4:
                            emit_av_I(pair, I, fillers)
                            done = True
                    if not done:
                        for f in fillers:
                            f()
                    for f in post16:
                        f()
                    if step == 16:
                        for j, tl in enumerate(range(28, 32)):
                            emit_outproj_tile(tl, ("s", "v")[j % 2])

    nc.finalize()
    return nc


def host_inputs(x, W_qkv, W_out, ln_g, ln_b):
    """Prepare per-core input maps (layout/sharding/dtype work plus
    weight-only algebra: ln_g fold, G = colsum(Wg), bq = ln_b @ Wg)."""
    import ml_dtypes
    bf16 = ml_dtypes.bfloat16
    fp8 = ml_dtypes.float8_e4m3
    x = np.asarray(x, dtype=np.float32)
    W_qkv = np.asarray(W_qkv, dtype=np.float32)
    W_out = np.asarray(W_out, dtype=np.float32)
    ln_g = np.asarray(ln_g, dtype=np.float32)
    ln_b = np.asarray(ln_b, dtype=np.float32)

    xt = np.ascontiguousarray(x.reshape(T, DIMK).T)       # [1024, 4096] f32
    xt_pm = np.ascontiguousarray(xt.reshape(KC, P, T).transpose(1, 0, 2))
    x8 = np.clip(xt_pm, -240, 240).astype(fp8)
    dx8 = (xt_pm - x8.astype(np.float32)).astype(fp8)
    xi8 = np.ascontiguousarray(np.stack([x8, dx8], axis=2))  # [P, KC, 2, T]

    Wg = W_qkv * ln_g[:, None]
    G = Wg.sum(axis=0) * WSCALE
    bq = (ln_b @ Wg) * WSCALE

    inv_freq = (1.0 / (10000.0 ** (np.arange(0, D, 2, dtype=np.float32) / D))).astype(np.float32)
    tpos = np.arange(NT, dtype=np.float32)
    freqs = np.outer(tpos, inv_freq).astype(np.float32)
    emb = np.concatenate([freqs, freqs], axis=1)
    cosT = np.cos(emb).T.astype(np.float32)
    sinT = np.sin(emb).T.astype(np.float32)
    ss_signed = np.concatenate([-sinT[:32], sinT[32:]], axis=0)
    cc = np.ascontiguousarray(np.tile(cosT, (2, 1)).astype(bf16))
    ss = np.ascontiguousarray(np.tile(ss_signed, (2, 1)).astype(bf16))
    tri = (np.arange(P)[None, :] >= np.arange(P)[:, None]).astype(bf16)
    perm = np.zeros((P, P), np.float32)
    for m in range(P):
        blk = (m // D) * D
        perm[blk + (m % D + 32) % D, m] = 1.0
    perm = perm.astype(bf16)

    in_maps = []
    for c in range(8):
        qs = slice(P * c, P * (c + 1))
        wl = np.concatenate([Wg[:, qs],
                             Wg[:, 1024 + P * c : 1024 + P * (c + 1)],
                             Wg[:, 2048 + P * c : 2048 + P * (c + 1)]],
                            axis=1) * WSCALE
        w8 = np.clip(wl, -240, 240).astype(fp8)
        dw8 = (wl - w8.astype(np.float32)).astype(fp8)
        # p-major chunked layout so one DMA loads the whole weight tile
        w8pm = np.ascontiguousarray(
            w8.reshape(KC, P, 384).transpose(1, 0, 2))
        dw8pm = np.ascontiguousarray(
            dw8.reshape(KC, P, 384).transpose(1, 0, 2))
        gsel = np.concatenate([G[qs], G[1024 + P * c : 1024 + P * (c + 1)],
                               G[2048 + P * c : 2048 + P * (c + 1)]])
        bsel = np.concatenate([bq[qs], bq[1024 + P * c : 1024 + P * (c + 1)],
                               bq[2048 + P * c : 2048 + P * (c + 1)]])
        gb = np.stack([bsel, -gsel]).astype(bf16)            # [2, 384]
        if OUTPROJ_FP8:
            wo_c = np.clip(W_out[qs, :] * 32.0, -240, 240)
            wo = np.ascontiguousarray(
                wo_c.reshape(2, D, DIMK).transpose(1, 0, 2).astype(fp8))
        else:
            wo = np.ascontiguousarray(W_out[qs, :].astype(bf16))
        in_maps.append({
            "xi8": xi8,
            "xs8": np.ascontiguousarray(
                x8[:, :, 512 * c : 512 * (c + 1)].reshape(P, KC * 512)),
            "w8": w8pm,
            "dw8": dw8pm,
            "wo": wo,
            "gb": gb,
            "cc": cc, "ss": ss, "tri": tri,
            "perm": perm,
        })
    return in_maps


_NC_CACHE = {}


def get_program():
    if "nc" not in _NC_CACHE:
        _NC_CACHE["nc"] = build_program()
    return _NC_CACHE["nc"]


LAST_RESULTS = {}


def kernel(x, W_qkv, W_out, b_out, ln_g, ln_b):
    import os
    from concourse.bass_utils import run_bass_kernel_spmd
    nc = get_program()
    in_maps = host_inputs(x, W_qkv, W_out, ln_g, ln_b)
    kw = {}
    if os.environ.get("BASS_KERNEL_TMPDIR"):
        kw["tmpdir"] = os.environ["BASS_KERNEL_TMPDIR"]
    if os.environ.get("BASS_KERNEL_TRACE"):
        kw["trace"] = True
    res = run_bass_kernel_spmd(nc, in_maps, list(range(8)), **kw)
    LAST_RESULTS["res"] = res
    total = np.zeros((T, DIMK), dtype=np.float32)
    for r in res.results:
        total += np.asarray(r["out"], dtype=np.float32)
    total += np.asarray(b_out, dtype=np.float32)[None, :]
    return total.reshape(2, NT, DIMK)
